# revision 9
# baseline (speedup 1.0000x reference)
#!/usr/bin/env python
"""2-layer GCN (AUGGCN) on 8 TRN2 NeuronCores via Bass/Tile.

Strategy (1D node partition, dst-routed edges):
- Nodes padded to 100352 = 8 cores x 12544 dsts = 4 src-ranges x 25088.
- Phase 1: each core computes y1 = dinv * (x @ W1) for its node slice
  (fp16 table rows, node-major), AllGather -> pair-shared full table.
- Layer aggregation (both layers identical): per-edge dma_gather of 256B
  table rows (int16 idx per src-range), one-hot S [128e x 128d] built on
  DVE (iota + is_equal), PE matmul S^T @ G accumulated per 128-dst PSUM
  window; rank-1 matmul adds sqrt(deg) (x) b1 so the ACT flush
  relu(dinv * psum) yields relu(dinv*agg + b1).
- z2 = dinv * relu(h1) is the layer-2 table: agg2 = A @ z2, then
  h2 = relu(dinv*agg2 @ W2 + b2) folded into per-window tensor_tensor_reduce
  against replicated W2 columns + a final fused head (relu, @Wc, sigmoid).
"""
import os
import sys

os.environ.setdefault("NEURON_SCRATCHPAD_PAGE_SIZE", "64")
sys.path.insert(0, "/opt/trn_rl_repo")

import numpy as np

import concourse.bacc as bacc
import concourse.mybir as mybir
import concourse.tile as tile
from concourse import bass_utils

dt = mybir.dt

NCORES = 8
NV = 100000          # real nodes
NFEAT_IN = 165
NFEAT = 128
DPC = 12544          # dsts per core (98 windows x 128)
WPC = 98             # windows per core
NRANGE = 4
RSIZE = 25088        # src range size (fits int16)
NTAB = NCORES * DPC  # 100352 = 4 * RSIZE
CALL = 1024          # idxs per dma_gather call


def _build_edge_meta(edge_index):
    """Host-side routing: group edges by (core, window, src-range); uniform
    per-(w,r) block budgets across cores (max over cores).

    Returns deg [NTAB], per-core idx streams (int16, range-concat, padded),
    per-core dstloc [128, NBLK] fp32, and the compile-time block geometry.
    """
    src = np.concatenate([edge_index[0], np.arange(NV, dtype=np.int64)]).astype(np.int64)
    dst = np.concatenate([edge_index[1], np.arange(NV, dtype=np.int64)]).astype(np.int64)
    E = src.shape[0]

    deg = np.bincount(dst, minlength=NTAB).astype(np.float32)
    deg[NV:] = 1.0

    core = dst // DPC
    w = (dst % DPC) // 128
    r = src // RSIZE
    dloc = (dst % 128).astype(np.float32)
    iloc = (src % RSIZE).astype(np.int16)

    key = ((core * WPC + w) * NRANGE + r).astype(np.int64)
    order = np.argsort(key, kind="stable")
    key_s = key[order]
    iloc_s = iloc[order]
    dloc_s = dloc[order]

    counts = np.bincount(key_s, minlength=NCORES * WPC * NRANGE).reshape(
        NCORES, WPC, NRANGE)
    B = np.ceil(counts.max(axis=0) / 128).astype(np.int64)  # [WPC, NRANGE]

    # positions of each (w, r) group inside its range stream (in blocks)
    pref = np.zeros((WPC, NRANGE), np.int64)
    for rr in range(NRANGE):
        pref[:, rr] = np.concatenate([[0], np.cumsum(B[:, rr])[:-1]])
    nblk_r = B.sum(axis=0)                      # blocks per range
    nr_r = nblk_r * 128                         # idx slots per range
    nr_r_pad = ((nr_r + CALL - 1) // CALL) * CALL
    blkbase = np.concatenate([[0], np.cumsum(nblk_r)[:-1]])  # block-id base/range
    NBLK = int(nblk_r.sum())
    NRtot = int(nr_r_pad.sum())
    coff = np.concatenate([[0], np.cumsum(nr_r_pad)[:-1]])   # idx col offsets (elems)

    # per-edge slot within its group
    gstart = np.zeros(NCORES * WPC * NRANGE + 1, np.int64)
    np.cumsum(np.bincount(key_s, minlength=NCORES * WPC * NRANGE), out=gstart[1:])
    slot_in_grp = np.arange(E, dtype=np.int64) - gstart[key_s]

    # stream position within range (in idx slots), then global arrays
    w_s = key_s // NRANGE % WPC
    r_s = key_s % NRANGE
    c_s = key_s // (NRANGE * WPC)
    pos = pref[w_s, r_s] * 128 + slot_in_grp     # slot within range stream
    gpos = coff[r_s] + pos                       # position in concatenated stream
    bid = blkbase[r_s] + pref[w_s, r_s] + slot_in_grp // 128
    lane = pos % 128

    idx_all = np.zeros((NCORES, NRtot), np.int16)
    dstloc_all = np.full((NCORES, 128, NBLK), -1.0, np.float32)
    idx_all[c_s, gpos] = iloc_s
    dstloc_all[c_s, lane, bid] = dloc_s

    # wrap idxs for dma_gather: idx j -> partition j%16, col j//16, tiled x8
    wrapped = idx_all.reshape(NCORES, NRtot // 16, 16).transpose(0, 2, 1)
    wrapped = np.tile(wrapped, (1, 8, 1)).copy()  # [NCORES, 128, NRtot//16]

    geom = dict(B=B, pref=pref, blkbase=blkbase, nr_r_pad=nr_r_pad,
                coff=coff, NBLK=NBLK, NRtot=NRtot)
    return deg, wrapped, dstloc_all, geom


def _build_program(geom):
    """Build the SPMD Bass program (uniform across cores)."""
    B = geom["B"]; pref = geom["pref"]; blkbase = geom["blkbase"]
    nr_r_pad = geom["nr_r_pad"]; coff = geom["coff"]
    NBLK = geom["NBLK"]; NRtot = geom["NRtot"]

    nc = bacc.Bacc("TRN2", target_bir_lowering=False, debug=False,
                   num_devices=NCORES, num_swdge_queues=4)

    f16, f32, i16 = dt.float16, dt.float32, dt.int16
    T = nc.dram_tensor
    t_xTa = T("xTa", [128, DPC], f16, kind="ExternalInput")
    t_xTb = T("xTb", [NFEAT_IN - 128, DPC], f16, kind="ExternalInput")
    t_W1a = T("W1a", [128, NFEAT], f16, kind="ExternalInput")
    t_W1b = T("W1b", [NFEAT_IN - 128, NFEAT], f16, kind="ExternalInput")
    t_b1 = T("b1r", [1, NFEAT], f32, kind="ExternalInput")
    t_degT = T("degT", [1, DPC], f32, kind="ExternalInput")
    t_degG = T("degG", [128, WPC], f32, kind="ExternalInput")
    t_idx = T("idx", [128, NRtot // 16], i16, kind="ExternalInput")
    t_dstloc = T("dstloc", [128, NBLK], f32, kind="ExternalInput")
    t_ident = T("ident", [128, 128], f16, kind="ExternalInput")
    t_W2cr = T("W2cr", [128, 2 * NFEAT], f16, kind="ExternalInput")
    t_b2pat = T("b2pat", [128, 2 * WPC], f32, kind="ExternalInput")
    t_Wcpat = T("Wcpat", [128, 2 * WPC], f32, kind="ExternalInput")
    t_bcr = T("bcr", [128, 1], f32, kind="ExternalInput")
    t_out = T("out", [DPC, 1], f32, kind="ExternalOutput")

    y1full = T("y1full", [NTAB, NFEAT], f16, addr_space="Shared")
    z2full = T("z2full", [NTAB, NFEAT], f16, addr_space="Shared")

    with tile.TileContext(nc) as tc:
        with tc.tile_pool(name="persist", bufs=1) as pp, \
             tc.tile_pool(name="dram", bufs=1, space="DRAM") as dram:

            # ---------- persistent loads ----------
            idx_t = pp.tile([128, NRtot // 16], i16)
            nc.sync.dma_start(out=idx_t[:], in_=t_idx.ap())
            dstloc_t = pp.tile([128, NBLK], f32)
            nc.sync.dma_start(out=dstloc_t[:], in_=t_dstloc.ap())
            iota_t = pp.tile([128, 128], i16)
            nc.gpsimd.iota(iota_t[:], pattern=[[1, 128]], base=0,
                           channel_multiplier=0)
            b1_t = pp.tile([1, NFEAT], f32)
            nc.sync.dma_start(out=b1_t[:], in_=t_b1.ap())
            W2cr_t = pp.tile([128, 2 * NFEAT], f16)
            nc.sync.dma_start(out=W2cr_t[:], in_=t_W2cr.ap())
            b2pat_t = pp.tile([128, 2 * WPC], f32)
            nc.sync.dma_start(out=b2pat_t[:], in_=t_b2pat.ap())
            Wcpat_t = pp.tile([128, 2 * WPC], f32)
            nc.sync.dma_start(out=Wcpat_t[:], in_=t_Wcpat.ap())
            bcr_t = pp.tile([128, 1], f32)
            nc.sync.dma_start(out=bcr_t[:], in_=t_bcr.ap())

            degT_t = pp.tile([1, DPC], f32)
            nc.sync.dma_start(out=degT_t[:], in_=t_degT.ap())
            dsqrtT_t = pp.tile([1, DPC], f32)
            nc.scalar.sqrt(dsqrtT_t[:], degT_t[:])
            degG_t = pp.tile([128, WPC], f32)
            nc.sync.dma_start(out=degG_t[:], in_=t_degG.ap())
            dsqrtG_t = pp.tile([128, WPC], f32)
            nc.scalar.sqrt(dsqrtG_t[:], degG_t[:])
            dinvG_t = pp.tile([128, WPC], f32)
            nc.vector.reciprocal(dinvG_t[:], dsqrtG_t[:])

            head_t = pp.tile([128, WPC, 2], f32)

            # ---------- phase 1: y1 slice ----------
            y1slice = dram.tile([DPC, NFEAT], f16)
            with tc.tile_pool(name="ph1", bufs=1) as p1p, \
                 tc.tile_pool(name="fpool", bufs=4) as fpool, \
                 tc.tile_pool(name="pone", bufs=2, space="PSUM") as pone:
                ident_t = p1p.tile([128, 128], f16)
                nc.sync.dma_start(out=ident_t[:], in_=t_ident.ap())
                W1a_t = p1p.tile([128, NFEAT], f16)
                nc.sync.dma_start(out=W1a_t[:], in_=t_W1a.ap())
                W1b_t = p1p.tile([NFEAT_IN - 128, NFEAT], f16)
                nc.sync.dma_start(out=W1b_t[:], in_=t_W1b.ap())
                xTa_t = p1p.tile([128, DPC], f16)
                nc.sync.dma_start(out=xTa_t[:], in_=t_xTa.ap())
                xTb_t = p1p.tile([NFEAT_IN - 128, DPC], f16)
                nc.sync.dma_start(out=xTb_t[:], in_=t_xTb.ap())

                for ch in range(0, DPC, 512):
                    cw = min(512, DPC - ch)
                    ps1 = pone.tile([128, 512], f32, tag="p1")
                    nc.tensor.matmul(out=ps1[:, :cw], lhsT=W1a_t[:],
                                     rhs=xTa_t[:, ch:ch + cw],
                                     start=True, stop=False)
                    nc.tensor.matmul(out=ps1[:, :cw], lhsT=W1b_t[:],
                                     rhs=xTb_t[:, ch:ch + cw],
                                     start=False, stop=True)
                    for t4 in range(cw // 128):
                        nb = ch + t4 * 128
                        a_sb = fpool.tile([128, 128], f16, tag="p1a")
                        nc.scalar.activation(a_sb[:], ps1[:, t4 * 128:(t4 + 1) * 128],
                                             mybir.ActivationFunctionType.Copy)
                        tr_ps = pone.tile([128, 128], f16, tag="p1t")
                        nc.tensor.transpose(tr_ps[:], a_sb[:], ident_t[:])
                        y_sb = fpool.tile([128, 128], f16, tag="p1y")
                        nc.vector.tensor_scalar_mul(y_sb[:], tr_ps[:],
                                                    dinvG_t[:, nb // 128:nb // 128 + 1])
                        nc.sync.dma_start(out=y1slice[nb:nb + 128, :], in_=y_sb[:])

            nc.gpsimd.collective_compute(
                "AllGather", mybir.AluOpType.bypass,
                replica_groups=[list(range(NCORES))],
                ins=[y1slice[:].opt()], outs=[y1full.ap().opt()],
            )

            _cm_pacc = tc.tile_pool(name="pacc", bufs=6, space="PSUM")
            pacc = _cm_pacc.__enter__()
            _cm_stage = tc.tile_pool(name="stage", bufs=3)
            _cm_spool = tc.tile_pool(name="spool", bufs=8)
            _cm_zpool = tc.tile_pool(name="zpool", bufs=4)
            stage = _cm_stage.__enter__()
            spool = _cm_spool.__enter__()
            zpool = _cm_zpool.__enter__()

            # ---------- aggregation layer ----------
            qn = [0]

            def agg_layer(table, flush_fn, layer):
                next_call = [0] * NRANGE
                gtiles = [dict() for _ in range(NRANGE)]

                def ensure_call(rr, c):
                    while next_call[rr] <= c:
                        k = next_call[rr]
                        g_t = stage.tile([128, CALL // 128, NFEAT], f16,
                                         tag=f"g{rr}")
                        col0 = (int(coff[rr]) + k * CALL) // 16
                        nc.gpsimd.dma_gather(
                            out_ap=g_t[:, :, :],
                            in_ap=table.ap()[rr * RSIZE:(rr + 1) * RSIZE, :],
                            idxs_ap=idx_t[:, col0:col0 + CALL // 16],
                            num_idxs=CALL, num_idxs_reg=CALL,
                            elem_size=NFEAT, single_packet=True,
                            queue_num=qn[0] % 4)
                        qn[0] += 1
                        gtiles[rr][k] = g_t
                        next_call[rr] += 1

                for w in range(WPC):
                    acc = pacc.tile([128, 128], f32, tag="acc")
                    first = True
                    for rr in range(NRANGE):
                        for b in range(int(B[w, rr])):
                            pos128 = int(pref[w, rr]) + b
                            c, slot = divmod(pos128, CALL // 128)
                            ensure_call(rr, c)
                            g_t = gtiles[rr][c]
                            bid = int(blkbase[rr]) + pos128
                            s_t = spool.tile([128, 128], f16, tag="s")
                            nc.vector.tensor_scalar(
                                out=s_t[:], in0=iota_t[:],
                                scalar1=dstloc_t[:, bid:bid + 1], scalar2=None,
                                op0=mybir.AluOpType.is_equal)
                            nc.tensor.matmul(out=acc[:], lhsT=s_t[:],
                                             rhs=g_t[:, slot, :],
                                             start=first, stop=False)
                            first = False
                    if layer == 1:
                        nc.tensor.matmul(out=acc[:],
                                         lhsT=dsqrtT_t[:, w * 128:(w + 1) * 128],
                                         rhs=b1_t[:], start=first, stop=True)
                    else:
                        # close the accumulation group (bias folded in head)
                        nc.tensor.matmul(out=acc[:],
                                         lhsT=dsqrtT_t[:, w * 128:(w + 1) * 128],
                                         rhs=zero1_t[:], start=first, stop=True)
                    flush_fn(w, acc)
                # drop tile refs
                for rr in range(NRANGE):
                    gtiles[rr].clear()

            zero1_t = pp.tile([1, NFEAT], f32)
            nc.vector.memset(zero1_t[:], 0.0)

            z2slice = dram.tile([DPC, NFEAT], f16)

            def flush_l1(w, acc):
                h1_sb = zpool.tile([128, 128], f16, tag="h1")
                nc.scalar.activation(h1_sb[:], acc[:],
                                     mybir.ActivationFunctionType.Relu,
                                     scale=dinvG_t[:, w:w + 1])
                z2_sb = zpool.tile([128, 128], f16, tag="z2")
                nc.vector.tensor_scalar_mul(z2_sb[:], h1_sb[:],
                                            dinvG_t[:, w:w + 1])
                nc.sync.dma_start(out=z2slice[w * 128:(w + 1) * 128, :],
                                  in_=z2_sb[:])

            agg_layer(y1full, flush_l1, layer=1)

            nc.gpsimd.collective_compute(
                "AllGather", mybir.AluOpType.bypass,
                replica_groups=[list(range(NCORES))],
                ins=[z2slice[:].opt()], outs=[z2full.ap().opt()],
            )


            def flush_l2(w, acc):
                for j in range(2):
                    scr = zpool.tile([128, 128], f32, tag="scr")
                    nc.vector.tensor_tensor(
                        out=scr[:], in0=acc[:],
                        in1=W2cr_t[:, j * NFEAT:(j + 1) * NFEAT],
                        op=mybir.AluOpType.mult)
                    nc.vector.tensor_reduce(
                        out=head_t[:, w, j:j + 1], in_=scr[:],
                        axis=mybir.AxisListType.X, op=mybir.AluOpType.add)

            agg_layer(z2full, flush_l2, layer=2)

            # ---------- head ----------
            hA = pp.tile([128, WPC, 2], f32)
            hB = pp.tile([128, WPC, 2], f32)
            nc.vector.tensor_tensor(
                out=hA[:, :, :], in0=head_t[:, :, :],
                in1=dinvG_t[:].unsqueeze(-1).broadcast_to([128, WPC, 2]),
                op=mybir.AluOpType.mult)
            nc.vector.tensor_tensor(
                out=hB[:, :, :], in0=hA[:, :, :],
                in1=b2pat_t[:].rearrange("p (w j) -> p w j", j=2),
                op=mybir.AluOpType.add)
            nc.vector.tensor_scalar_max(hA[:, :, :], hB[:, :, :], 0.0)
            nc.vector.tensor_tensor(
                out=hB[:, :, :], in0=hA[:, :, :],
                in1=Wcpat_t[:].rearrange("p (w j) -> p w j", j=2),
                op=mybir.AluOpType.mult)
            o_t = pp.tile([128, WPC], f32)
            nc.vector.tensor_reduce(out=o_t[:], in_=hB[:, :, :],
                                    axis=mybir.AxisListType.X,
                                    op=mybir.AluOpType.add)
            o_b = pp.tile([128, WPC], f32)
            nc.vector.tensor_scalar_add(o_b[:], o_t[:], bcr_t[:, 0:1])
            o_s = pp.tile([128, WPC], f32)
            nc.scalar.activation(o_s[:], o_b[:],
                                 mybir.ActivationFunctionType.Sigmoid)
            nc.sync.dma_start(
                out=t_out.ap().rearrange("(w p) one -> p (w one)", p=128),
                in_=o_s[:, :])

            _cm_zpool.__exit__(None, None, None)
            _cm_spool.__exit__(None, None, None)
            _cm_stage.__exit__(None, None, None)
            _cm_pacc.__exit__(None, None, None)

    nc.compile()
    return nc


def _build_inputs(x, edge_index, W1, b1, W2, b2, Wc, bc, deg, wrapped, dstloc):
    xp = np.zeros((NTAB, NFEAT_IN), np.float16)
    xp[:NV] = x.astype(np.float16)
    xT = xp.T.copy()  # [165, NTAB]
    W1h = W1.astype(np.float16)
    ident = np.eye(128, dtype=np.float16)
    W2cr = np.concatenate([np.tile(W2[:, 0].astype(np.float16), (128, 1)),
                           np.tile(W2[:, 1].astype(np.float16), (128, 1))],
                          axis=1)
    b2pat = np.tile(b2.astype(np.float32), (128, WPC))
    Wcpat = np.tile(Wc[:, 0].astype(np.float32), (128, WPC))
    bcr = np.full((128, 1), float(bc[0]), np.float32)
    b1r = b1.astype(np.float32).reshape(1, NFEAT)

    in_maps = []
    for c in range(NCORES):
        sl = slice(c * DPC, (c + 1) * DPC)
        degsl = deg[sl]
        in_maps.append({
            "xTa": xT[:128, sl].copy(),
            "xTb": xT[128:, sl].copy(),
            "W1a": W1h[:128], "W1b": W1h[128:],
            "b1r": b1r,
            "degT": degsl.reshape(1, DPC),
            "degG": degsl.reshape(WPC, 128).T.copy(),
            "idx": wrapped[c],
            "dstloc": dstloc[c],
            "ident": ident,
            "W2cr": W2cr,
            "b2pat": b2pat,
            "Wcpat": Wcpat,
            "bcr": bcr,
        })
    return in_maps


_CACHE = {}


def kernel(x, edge_index, W1, b1, W2, b2, Wc, bc, _trace=False):
    deg, wrapped, dstloc, geom = _build_edge_meta(edge_index)
    gkey = (geom["NBLK"], geom["NRtot"], geom["B"].tobytes())
    if gkey in _CACHE:
        nc = _CACHE[gkey]
    else:
        nc = _build_program(geom)
        _CACHE[gkey] = nc
    in_maps = _build_inputs(x, edge_index, W1, b1, W2, b2, Wc, bc,
                            deg, wrapped, dstloc)
    res = bass_utils.run_bass_kernel_spmd(
        nc, in_maps, core_ids=list(range(NCORES)), trace=_trace)
    out = np.concatenate([res.results[c]["out"] for c in range(NCORES)], axis=0)
    out = out[:NV]
    kernel.last_exec_time_ns = res.exec_time_ns
    return out.astype(np.float32)


kernel.last_exec_time_ns = None
